# revision 25
# baseline (speedup 1.0000x reference)
"""Multi-head attention (B=2, S=2048, D=1024, H=16, causal) on 8 Trainium2
NeuronCores via Bass/Tile.

Sharding: core c -> batch c//4, heads [4*(c%4), 4*(c%4)+4)  (DP over batch x
TP over heads).  QKV weights column-parallel, O row-parallel; the 4 partial
[S, D] outputs per batch are summed on the host (gather step), bias bo added
there too.

Per-core dataflow (bf16 matmuls, fp32 PSUM accumulation):
  - host supplies x.T per batch in a dc-swizzled layout ([p, dc, s], 16KB
    contiguous per partition) so input DMA runs at full bandwidth.
  - Q/K proj -> qT/kT [dk_c=256, S] (head-major, 2 chunks of 128 = 2 heads);
    V proj -> natural [S, 260]: per head 64 cols of V plus a ones column for
    the flash-style softmax denominator.
  - scores transposed: sT[k, q] = kT.T @ qT per head; causal masking is done
    ON THE PE: diagonal 128-blocks get a second accumulating matmul
    (identity.T @ Tneg) that adds -1e9 to the strictly-upper triangle, so
    exp() zeroes it with no vector-engine hop; strictly-upper blocks are
    skipped and diagonal-band matmuls are column-trimmed.
  - exp on ScalarE reads scores PSUM directly; attn@V: outT[65, q] +=
    V'[k,65].T @ expT[k,q]; row 64 accumulates the denominator.
  - normalize: DVE reciprocal of the denominator row + PE outer-product
    broadcast + Pool-engine multiply into head-PAIR tiles [128, S], so the
    O projection contracts 128 partitions (2 heads x 64 dk) per matmul.
  - PSUM -> f16 SBUF -> DRAM f16 partials.

Schedule: PAIR-MAJOR with filler injection.  Prologue: K proj (pair 0) +
Q proj (pair 0, S-half 0) while DMAs stream.  Attention for pair 0 then
starts ~25us in; the remaining projection work (V per-kc units, pair-1 K/Q,
pair-0 Q half 1) is injected one unit at a time between attention groups so
the PE never idles while ScalarE exps drain.  During pair 1's attention the
fillers are pair-0's bc/normalize and the O projection of finished q-chunks.
Attention groups are software-pipelined one group deep (attn@V of group g-1
issues after scores+exp of group g).
"""

import os
import sys
import types
from collections import deque

import numpy as np

B, S, D, H = 2, 2048, 1024, 16
DK = D // H  # 64
N_CORES = 8
HPC = 4  # heads per core
SCALE = 1.0 / np.sqrt(np.float32(DK))  # folded into Wq/bq on host

QC = 512  # query block (free dim of scores matmuls)
NQC = S // QC  # 4
GK = 2  # key blocks per exp group -> scores psum tile [128, GK, QC]


def _install_ntff_hook():
    """The image's antenv lacks axon_hooks; register the NTFF profile hook
    ourselves so run_bass_kernel_spmd(trace=True) works."""
    if "antenv.axon_hooks" in sys.modules:
        return
    try:
        mod = types.ModuleType("antenv.axon_hooks")
        state = {"hook": None}
        mod.set_axon_ntff_profile_hook = lambda h: state.__setitem__("hook", h)
        mod.get_axon_ntff_profile_hook = lambda: state["hook"]
        sys.modules["antenv.axon_hooks"] = mod
        from trn_agent_boot.trn_boot import _ntff_profile_via_ctypes

        mod.set_axon_ntff_profile_hook(
            _ntff_profile_via_ctypes("/opt/axon/libaxon_pjrt.so")
        )
    except Exception:
        sys.modules.pop("antenv.axon_hooks", None)


def _split_multi_waits(nc):
    """This walrus build accepts at most ONE sem wait per instruction; Tile
    packs several.  Split extras into preceding single-wait NOPs on the same
    engine (equivalent semantics: the engine blocks on them in order)."""
    import bass_rust

    cnt = 0
    for bbw in nc.main_func.blocks:
        bb = bbw.bb if hasattr(bbw, "bb") else bbw
        out = []
        changed = False
        for ins in bb.instructions:
            si = ins.sync_info
            if si is not None and len(si.on_wait) > 1:
                changed = True
                waits = list(si.on_wait)
                for w in waits[:-1]:
                    cnt += 1
                    nop = bass_rust.InstNoOp(name=f"I-wsp{cnt}", ins=[], outs=[])
                    nop.engine = ins.engine
                    nop.sync_info = bass_rust.SyncInfo(on_wait=[w], on_update=[])
                    out.append(nop)
                si.on_wait = [waits[-1]]
                ins.sync_info = si
            out.append(ins)
        if changed:
            bb.instructions = out
    return cnt


def _build_nc(split=True, phase=5):
    from contextlib import ExitStack

    import concourse.bass as bass
    import concourse.tile as tile
    from concourse import mybir

    bf16 = mybir.dt.bfloat16
    f16 = mybir.dt.float16
    f32 = mybir.dt.float32

    nc = bass.Bass()
    xq_h = [
        nc.declare_dram_parameter(f"xq{i}", [128, 8, 1024], bf16, isOutput=False)
        for i in range(2)
    ]
    xk_h = [
        nc.declare_dram_parameter(f"xk{i}", [128, 8, 1024], bf16, isOutput=False)
        for i in range(2)
    ]
    xv_h = [
        nc.declare_dram_parameter(f"xv{i}", [128, 8, 1024], bf16, isOutput=False)
        for i in range(2)
    ]
    wq = nc.declare_dram_parameter("wq", [128, 8 * 256], bf16, isOutput=False)
    wk = nc.declare_dram_parameter("wk", [128, 8 * 256], bf16, isOutput=False)
    wv = nc.declare_dram_parameter("wv", [128, 8 * 260], bf16, isOutput=False)
    wo = nc.declare_dram_parameter("wo", [128, 2 * 1024], bf16, isOutput=False)
    bq = nc.declare_dram_parameter("bq", [128, 2], f32, isOutput=False)
    bk = nc.declare_dram_parameter("bk", [128, 2], f32, isOutput=False)
    bvp = nc.declare_dram_parameter("bvp", [1, 260], f32, isOutput=False)
    identm = nc.declare_dram_parameter("identm", [128, 128], bf16, isOutput=False)
    tneg = nc.declare_dram_parameter("tneg", [128, 128], bf16, isOutput=False)
    outp = nc.declare_dram_parameter("outp", [S, D], f16, isOutput=True)

    with tile.TileContext(nc) as tc, ExitStack() as ctx:
        consts = ctx.enter_context(tc.tile_pool(name="consts", bufs=1))
        xs = ctx.enter_context(tc.tile_pool(name="xs", bufs=6))
        acts = ctx.enter_context(tc.tile_pool(name="acts", bufs=1))
        exps = ctx.enter_context(tc.tile_pool(name="exps", bufs=6))
        rcps = ctx.enter_context(tc.tile_pool(name="rcps", bufs=4))
        osb = ctx.enter_context(tc.tile_pool(name="osb", bufs=4))
        ps_small = ctx.enter_context(
            tc.tile_pool(name="ps_small", bufs=2, space="PSUM")
        )
        ps_sc = ctx.enter_context(tc.tile_pool(name="ps_sc", bufs=2, space="PSUM"))
        ps_av = ctx.enter_context(tc.tile_pool(name="ps_av", bufs=2, space="PSUM"))

        # ---- persistent activation tiles ----
        qt = [acts.tile([128, S], bf16, name=f"qt{m}", tag=f"qt{m}") for m in range(2)]
        kt = [acts.tile([128, S], bf16, name=f"kt{m}", tag=f"kt{m}") for m in range(2)]
        vh_sb = acts.tile([128, 16, 260], bf16, name="vh", tag="vh")
        # attn-out as head PAIRS [2 heads x 64 dk = 128 partitions, S]
        outT = [
            acts.tile([128, S], bf16, name=f"outT{p}", tag=f"outT{p}")
            for p in range(2)
        ]

        def dma2(dst_tile, src):
            """Split a [128, 8, 1024] tensor load into two 1MB DMAs so the
            first accumulation chains can start earlier."""
            nc.sync.dma_start(out=dst_tile[:, 0:4, :], in_=src[:, 0:4, :])
            nc.sync.dma_start(out=dst_tile[:, 4:8, :], in_=src[:, 4:8, :])

        # ---- PE warm-up ----
        # The PE clock throttles when idle and needs ~3us of continuous work
        # to reach 2.4GHz.  While the first input DMAs stream in, run dummy
        # matmuls on a memset tile (never read) so the projections start at
        # full clock instead of ramping through them.
        scr = consts.tile([128, 512], bf16, name="scr")
        nc.vector.memset(scr[:], 0.0078125)
        warm_ps = ps_small.tile([128, 512], f32, name="ps", tag="ps")

        def warm(n):
            for _ in range(n):
                nc.tensor.matmul(
                    warm_ps[:],
                    lhsT=scr[:, 0:128],
                    rhs=scr[:],
                    start=True,
                    stop=True,
                    skip_group_check=True,
                )

        # ---- DMA stream (order == consume order) ----
        wk_sb = consts.tile([128, 8 * 256], bf16)
        nc.sync.dma_start(out=wk_sb[:], in_=wk[:])
        xk_t = []
        for half in range(2):
            t = xs.tile([128, 8, S // 2], bf16, name="xt", tag="xt")
            dma2(t, xk_h[half])
            xk_t.append(t)
        bk_sb = consts.tile([128, 2], f32)
        nc.sync.dma_start(out=bk_sb[:], in_=bk[:])
        wq_sb = consts.tile([128, 8 * 256], bf16, name="wq_sb")
        nc.sync.dma_start(out=wq_sb[:], in_=wq[:])
        bq_sb = consts.tile([128, 2], f32, name="bq_sb")
        nc.sync.dma_start(out=bq_sb[:], in_=bq[:])
        xq_t = [xs.tile([128, 8, S // 2], bf16, name="xt", tag="xt") for _ in range(2)]
        dma2(xq_t[0], xq_h[0])
        id_sb = consts.tile([128, 128], bf16, name="id_sb")
        nc.sync.dma_start(out=id_sb[:], in_=identm[:])
        tn_sb = consts.tile([128, 128], bf16, name="tn_sb")
        nc.sync.dma_start(out=tn_sb[:], in_=tneg[:])
        wv_sb = consts.tile([128, 8 * 260], bf16, name="wv_sb")
        nc.sync.dma_start(out=wv_sb[:], in_=wv[:])
        bvp_sb = consts.tile([128, 260], f32, name="bvp_sb")
        nc.sync.dma_start(out=bvp_sb[:], in_=bvp[:].to_broadcast((128, 260)))
        xv_t = [xs.tile([128, 8, S // 2], bf16, name="xt", tag="xt") for _ in range(2)]
        dma2(xv_t[0], xv_h[0])
        dma2(xq_t[1], xq_h[1])
        dma2(xv_t[1], xv_h[1])
        wo_sb = consts.tile([128, 2 * 1024], bf16, name="wo_sb")
        nc.sync.dma_start(out=wo_sb[:], in_=wo[:])
        ones_sb = consts.tile([65, 64], bf16)
        nc.vector.memset(ones_sb[:], 1.0)

        # ---- projection unit emitters ----
        def kq_unit(xt, wsb, bsb, dst, m, half, scq):
            """One [128,512] output chain of a K/Q projection."""
            sc = half * 2 + scq
            ps = ps_small.tile([128, 512], f32, name="ps", tag="ps")
            for dc in range(8):
                nc.tensor.matmul(
                    ps[:],
                    lhsT=wsb[:, dc * 256 + m * 128: dc * 256 + (m + 1) * 128],
                    rhs=xt[half][:, dc, scq * 512:(scq + 1) * 512],
                    start=(dc == 0),
                    stop=(dc == 7),
                )
            nc.vector.tensor_scalar_add(
                dst[m][:, sc * 512:(sc + 1) * 512], ps[:], bsb[:, m:m + 1]
            )

        def v_unit(st):
            """One s-block (= one kc block) of the V projection."""
            ps = ps_small.tile([128, 512], f32, name="ps", tag="ps")
            for dc in range(8):
                nc.tensor.matmul(
                    ps[:, :260],
                    lhsT=xv_t[st // 8][:, dc, (st % 8) * 128:(st % 8 + 1) * 128],
                    rhs=wv_sb[:, dc * 260:(dc + 1) * 260],
                    start=(dc == 0),
                    stop=(dc == 7),
                )
            nc.vector.tensor_add(vh_sb[:, st, :], ps[:, :260], bvp_sb[:])

        # ---- prologue: K proj pair0 (both halves) + Q proj pair0 half0 ----
        if phase >= 1:
            warm(12)
            for half in range(2):
                for scq in range(2):
                    kq_unit(xk_t, wk_sb, bk_sb, kt, 0, half, scq)
                    warm(3)
            for scq in range(2):
                kq_unit(xq_t, wq_sb, bq_sb, qt, 0, 0, scq)
                warm(3)

        # ---- filler queue for pair-0 attention ----
        fillers = deque()
        if phase >= 1:
            for st in (0, 1, 2, 3):
                fillers.append(lambda st=st: v_unit(st))
            for scq in range(2):
                fillers.append(
                    lambda scq=scq: kq_unit(xq_t, wq_sb, bq_sb, qt, 0, 1, scq)
                )
            for st in (4, 5, 6, 7):
                fillers.append(lambda st=st: v_unit(st))
            for half in range(2):
                for scq in range(2):
                    fillers.append(
                        lambda half=half, scq=scq: kq_unit(
                            xk_t, wk_sb, bk_sb, kt, 1, half, scq
                        )
                    )
            for st in (8, 9, 10, 11):
                fillers.append(lambda st=st: v_unit(st))
            for half in range(2):
                for scq in range(2):
                    fillers.append(
                        lambda half=half, scq=scq: kq_unit(
                            xq_t, wq_sb, bq_sb, qt, 1, half, scq
                        )
                    )
            for st in (12, 13, 14, 15):
                fillers.append(lambda st=st: v_unit(st))

        def inject(n):
            for _ in range(n):
                if not fillers:
                    return
                fillers.popleft()()

        # ---- attention ----
        tails = {}  # (pair, qc) -> (posb{h}, rcp{h})

        def trim_c0(qc, kc):
            jr = kc - 4 * qc
            return 128 * jr if jr >= 0 else 0

        def emit_scores_exp(qc, pair, g, exg):
            heads = (2 * pair, 2 * pair + 1)
            for h in heads:
                hr = slice(64 * (h % 2), 64 * (h % 2) + 64)
                pss = ps_sc.tile([128, GK, QC], f32, name="pss", tag="pss")
                for j in range(GK):
                    kc = GK * g + j
                    c0 = trim_c0(qc, kc)
                    diag = kc - 4 * qc >= 0
                    nc.tensor.matmul(
                        pss[:, j, c0:],
                        lhsT=kt[pair][hr, kc * 128:(kc + 1) * 128],
                        rhs=qt[pair][hr, qc * QC + c0:(qc + 1) * QC],
                        start=True,
                        stop=not diag,
                        skip_group_check=True,
                    )
                    if diag:
                        # add -1e9 to the strictly-upper triangle of the
                        # diagonal 128-block: psum += I.T @ Tneg (53ns)
                        nc.tensor.matmul(
                            pss[:, j, c0:c0 + 128],
                            lhsT=id_sb[:],
                            rhs=tn_sb[:],
                            start=False,
                            stop=True,
                            skip_group_check=True,
                        )
                ex = exps.tile([128, GK, QC], bf16, name="ex", tag="ex")
                # full-width exp (2-D contiguous AP); the trimmed-away columns
                # hold stale PSUM whose exp lands in ex columns the (equally
                # trimmed) attn@V matmuls never read.
                nc.scalar.activation(
                    ex[:, :, :], pss[:, :, :], mybir.ActivationFunctionType.Exp
                )
                exg[h] = ex

        def emit_attnv(qc, pair, g, po, last_kc, exg):
            for h in (2 * pair, 2 * pair + 1):
                for j in range(GK):
                    kc = GK * g + j
                    c0 = trim_c0(qc, kc)
                    nc.tensor.matmul(
                        po[h][:, c0:],
                        lhsT=vh_sb[:, kc, h * 65:(h + 1) * 65],
                        rhs=exg[h][:, j, c0:],
                        start=(kc == 0),
                        stop=(kc == last_kc),
                        skip_group_check=True,
                    )

        def emit_pair_tail(qc, pair, po):
            # stage attn-out to SBUF bf16 FIRST so the po PSUM banks free for
            # the next q-chunk with no long ops ahead of them; denominator
            # reciprocal then runs on ScalarE as exp(-ln x) (same table set
            # as the attention exps).
            posb_d, rcp_d = {}, {}
            for h in (2 * pair, 2 * pair + 1):
                posb = rcps.tile([64, 512], bf16, name="posb", tag="posb", bufs=8)
                with nc.allow_low_precision(reason="attn-out staged bf16"):
                    nc.vector.tensor_copy(posb[:, :], po[h][0:64, :])
                posb_d[h] = posb
            for h in (2 * pair, 2 * pair + 1):
                lg = rcps.tile([65, 512], f32, name="lg", tag="lg", bufs=4)
                nc.scalar.activation(
                    lg[64:65, :],
                    po[h][64:65, :],
                    mybir.ActivationFunctionType.Ln,
                )
                rcp = rcps.tile([65, 512], bf16, name="rcp", tag="rcp", bufs=8)
                nc.scalar.activation(
                    rcp[64:65, :],
                    lg[64:65, :],
                    mybir.ActivationFunctionType.Exp,
                    scale=-1.0,
                )
                rcp_d[h] = rcp
            tails[(pair, qc)] = (posb_d, rcp_d)

        def emit_bc(qc, pair):
            posb_d, rcp_d = tails[(pair, qc)]
            for h in (2 * pair, 2 * pair + 1):
                bc = ps_small.tile([128, 512], f32, name="ps", tag="ps")
                nc.tensor.matmul(
                    bc[0:64, :],
                    lhsT=ones_sb[64:65, :],
                    rhs=rcp_d[h][64:65, :],
                    start=True,
                    stop=True,
                )
                # DVE can't take two PSUM operands; stage via SBUF, then the
                # normalize multiply runs on the otherwise-idle Pool engine.
                bcs = rcps.tile([64, 512], bf16, name="bcs", tag="bcs", bufs=8)
                nc.vector.tensor_copy(bcs[:, :], bc[0:64, :])
                nc.gpsimd.tensor_mul(
                    outT[h // 2][64 * (h % 2):64 * (h % 2) + 64,
                                 qc * QC:(qc + 1) * QC],
                    posb_d[h][:, :],
                    bcs[:, :],
                )

        def oproj_unit(qc, sti):
            st = qc * 4 + sti
            for ns in range(2):
                ps = ps_small.tile([128, 512], f32, name="ps", tag="ps")
                for hp in range(2):
                    nc.tensor.matmul(
                        ps[:],
                        lhsT=outT[hp][:, st * 128:(st + 1) * 128],
                        rhs=wo_sb[:, hp * 1024 + ns * 512: hp * 1024 + (ns + 1) * 512],
                        start=(hp == 0),
                        stop=(hp == 1),
                    )
                ot = osb.tile([128, 512], f16, name="ot", tag="ot")
                with nc.allow_low_precision(reason="f16 partials"):
                    nc.vector.tensor_copy(ot[:], ps[:])
                nc.sync.dma_start(
                    out=outp[st * 128:(st + 1) * 128, ns * 512:(ns + 1) * 512],
                    in_=ot[:],
                )

        if phase >= 3:
            for pair in range(2):
                for qc in range(NQC):
                    po = {}
                    for h in (2 * pair, 2 * pair + 1):
                        po[h] = ps_av.tile([65, 512], f32, name="po", tag="po")
                    ngroups = GK * (qc + 1)
                    last_kc = 4 * qc + 3
                    prev = None
                    for g in range(ngroups):
                        exg = {}
                        emit_scores_exp(qc, pair, g, exg)
                        inject(2)
                        if prev is not None:
                            emit_attnv(qc, pair, prev[0], po, last_kc, prev[1])
                        prev = (g, exg)
                    emit_attnv(qc, pair, prev[0], po, last_kc, prev[1])
                    emit_pair_tail(qc, pair, po)
                    if pair == 1 and phase >= 4:
                        # both pairs of this qc are done: normalize pair1 now,
                        # queue this qc's oproj as fillers for the next chunk
                        emit_bc(qc, 1)
                        if phase >= 5:
                            for sti in range(4):
                                fillers.append(
                                    lambda qc=qc, sti=sti: oproj_unit(qc, sti)
                                )
                if pair == 0 and phase >= 4:
                    # while pair1's attention runs, normalize pair0's heads
                    for qc in range(NQC):
                        fillers.append(lambda qc=qc: emit_bc(qc, 0))
            # drain remaining fillers (late oprojs)
            inject(len(fillers))

        if phase < 5:
            ot = osb.tile([128, 512], f16, name="ot", tag="ot")
            nc.vector.memset(ot[:], 0.0)
            nc.sync.dma_start(out=outp[0:128, 0:512], in_=ot[:])

    if split:
        _split_multi_waits(nc)
    return nc


_NC_CACHE = None


def _get_nc():
    global _NC_CACHE
    if _NC_CACHE is None:
        _NC_CACHE = _build_nc()
    return _NC_CACHE


def _swizzle_w(wT, block):
    """wT [D, C] -> [128, 8*C] so that out[p, dc*C + j] = wT[dc*128 + p, j]."""
    dcs = wT.shape[0] // 128
    return np.ascontiguousarray(
        wT.reshape(dcs, 128, wT.shape[1]).transpose(1, 0, 2).reshape(128, -1)
    )


def _np_reference(q, k, v, mask, Wq, bq, Wk, bk, Wv, bv, Wo, bo):
    def split_heads(x):
        b, s, _ = x.shape
        return x.reshape(b, s, H, DK).transpose(0, 2, 1, 3)

    qh = split_heads(q @ Wq.T + bq)
    kh = split_heads(k @ Wk.T + bk)
    vh = split_heads(v @ Wv.T + bv)
    scores = np.einsum("bhqd,bhkd->bhqk", qh, kh) / np.sqrt(np.float32(DK))
    scores = np.where(mask, np.float32(-1e9), scores)
    scores = scores - scores.max(axis=-1, keepdims=True)
    e = np.exp(scores)
    attn = e / e.sum(axis=-1, keepdims=True)
    out = np.einsum("bhqk,bhkd->bhqd", attn, vh)
    out = out.transpose(0, 2, 1, 3).reshape(q.shape[0], -1, D)
    return (out @ Wo.T + bo).astype(np.float32)


def kernel(q, k, v, mask, Wq, bq, Wk, bk, Wv, bv, Wo, bo):
    import ml_dtypes

    bf16 = ml_dtypes.bfloat16

    q = np.asarray(q, np.float32)
    k = np.asarray(k, np.float32)
    v = np.asarray(v, np.float32)
    mask = np.asarray(mask, bool)
    Wq = np.asarray(Wq, np.float32)
    bq = np.asarray(bq, np.float32)
    Wk = np.asarray(Wk, np.float32)
    bk = np.asarray(bk, np.float32)
    Wv = np.asarray(Wv, np.float32)
    bv = np.asarray(bv, np.float32)
    Wo = np.asarray(Wo, np.float32)
    bo = np.asarray(bo, np.float32)

    causal = np.triu(np.ones((S, S), dtype=bool), k=1)
    if not np.array_equal(mask.reshape(S, S), causal):
        return _np_reference(q, k, v, mask, Wq, bq, Wk, bk, Wv, bv, Wo, bo)

    _install_ntff_hook()
    from concourse.bass_utils import run_bass_kernel_spmd

    nc = _get_nc()

    kk = np.arange(128)[:, None]
    qq = np.arange(128)[None, :]
    tneg_m = np.where(kk > qq, np.float32(-1e9), np.float32(0)).astype(bf16)
    ident_m = np.eye(128, dtype=np.float32).astype(bf16)

    # x.T [D, S] -> per half [128, 8, 1024] with x_h[p, dc, s] =
    # xT[dc*128 + p, half*1024 + s]; contiguous 16KB per partition.
    xT = {}
    for name, x in (("q", q), ("k", k), ("v", v)):
        per_b = []
        for b in range(B):
            xt = x[b].T.astype(bf16).reshape(8, 128, 2048)
            per_b.append(
                [
                    np.ascontiguousarray(
                        xt[:, :, hf * 1024:(hf + 1) * 1024].transpose(1, 0, 2)
                    )
                    for hf in range(2)
                ]
            )
        xT[name] = per_b

    in_maps = []
    for c in range(N_CORES):
        b = c // 4
        g = c % 4
        hs = slice(g * HPC * DK, (g + 1) * HPC * DK)  # 256 rows of W, cols of Wo
        wq_c = _swizzle_w((SCALE * Wq[hs]).T.astype(bf16), 256)
        wk_c = _swizzle_w(Wk[hs].T.astype(bf16), 256)
        # V' with a zero weight column at h*65+64 (ones come via bias row)
        wvT = Wv[hs].T  # [1024, 256]
        wvp = np.zeros((D, 260), np.float32)
        for h in range(HPC):
            wvp[:, h * 65:h * 65 + 64] = wvT[:, h * 64:(h + 1) * 64]
        wv_c = _swizzle_w(wvp.astype(bf16), 260)
        # wo: (Wo.T)[hs, :] [256, 1024] -> head-pair blocks [128, 2*1024]
        woT = np.ascontiguousarray(Wo[:, hs].T)
        wo_c = np.ascontiguousarray(
            woT.reshape(2, 128, 1024).transpose(1, 0, 2).reshape(128, 2048)
        ).astype(bf16)
        bq_c = np.ascontiguousarray(
            (SCALE * bq[hs]).reshape(2, 128).T.astype(np.float32)
        )
        bk_c = np.ascontiguousarray(bk[hs].reshape(2, 128).T.astype(np.float32))
        bvp_c = np.zeros((1, 260), np.float32)
        for h in range(HPC):
            bvp_c[0, h * 65:h * 65 + 64] = bv[hs][h * 64:(h + 1) * 64]
            bvp_c[0, h * 65 + 64] = 1.0
        in_maps.append(
            {
                "xq0": xT["q"][b][0],
                "xq1": xT["q"][b][1],
                "xk0": xT["k"][b][0],
                "xk1": xT["k"][b][1],
                "xv0": xT["v"][b][0],
                "xv1": xT["v"][b][1],
                "wq": wq_c,
                "wk": wk_c,
                "wv": wv_c,
                "wo": wo_c,
                "bq": bq_c,
                "bk": bk_c,
                "bvp": bvp_c,
                "identm": ident_m,
                "tneg": tneg_m,
            }
        )

    trace = bool(os.environ.get("BASSMHA_TRACE"))
    res = run_bass_kernel_spmd(nc, in_maps, list(range(N_CORES)), trace=trace)
    kernel._last_exec_ns = res.exec_time_ns
    kernel._last_mean_exec_ns = res.mean_exec_time_ns

    out = np.zeros((B, S, D), np.float64)
    for c in range(N_CORES):
        out[c // 4] += res.results[c]["outp"].astype(np.float64)
    out += bo.astype(np.float64)
    return out.astype(np.float32)


# revision 28
# speedup vs baseline: 1.0384x; 1.0384x over previous
"""Multi-head attention (B=2, S=2048, D=1024, H=16, causal) on 8 Trainium2
NeuronCores via Bass/Tile.

Sharding: core c -> batch c//4, heads [4*(c%4), 4*(c%4)+4)  (DP over batch x
TP over heads).  QKV weights column-parallel, O row-parallel; the 4 partial
[S, D] outputs per batch are summed on the host (gather step), bias bo added
there too.

Per-core dataflow (bf16 matmuls, fp32 PSUM accumulation):
  - host supplies x.T per batch in a dc-swizzled layout ([p, dc, s], 16KB
    contiguous per partition) so input DMA runs at full bandwidth.
  - Q/K proj -> qT/kT [dk_c=256, S] (head-major, 2 chunks of 128 = 2 heads);
    V proj -> natural [S, 260]: per head 64 cols of V plus a ones column for
    the flash-style softmax denominator.
  - scores transposed: sT[k, q] = kT.T @ qT per head; causal masking is done
    ON THE PE: diagonal 128-blocks get a second accumulating matmul
    (identity.T @ Tneg) that adds -1e9 to the strictly-upper triangle, so
    exp() zeroes it with no vector-engine hop; strictly-upper blocks are
    skipped and diagonal-band matmuls are column-trimmed.
  - exp on ScalarE reads scores PSUM directly; attn@V: outT[65, q] +=
    V'[k,65].T @ expT[k,q]; row 64 accumulates the denominator.
  - normalize: DVE reciprocal of the denominator row + PE outer-product
    broadcast + Pool-engine multiply into head-PAIR tiles [128, S], so the
    O projection contracts 128 partitions (2 heads x 64 dk) per matmul.
  - PSUM -> f16 SBUF -> DRAM f16 partials.

Schedule: PAIR-MAJOR with filler injection.  Prologue: K proj (pair 0) +
Q proj (pair 0, S-half 0) while DMAs stream.  Attention for pair 0 then
starts ~25us in; the remaining projection work (V per-kc units, pair-1 K/Q,
pair-0 Q half 1) is injected one unit at a time between attention groups so
the PE never idles while ScalarE exps drain.  During pair 1's attention the
fillers are pair-0's bc/normalize and the O projection of finished q-chunks.
Attention groups are software-pipelined one group deep (attn@V of group g-1
issues after scores+exp of group g).
"""

import os
import sys
import types
from collections import deque

import numpy as np

B, S, D, H = 2, 2048, 1024, 16
DK = D // H  # 64
N_CORES = 8
HPC = 4  # heads per core
SCALE = 1.0 / np.sqrt(np.float32(DK))  # folded into Wq/bq on host

QC = 512  # query block (free dim of scores matmuls)
NQC = S // QC  # 4
GK = 2  # key blocks per exp group -> scores psum tile [128, GK, QC]


def _install_ntff_hook():
    """The image's antenv lacks axon_hooks; register the NTFF profile hook
    ourselves so run_bass_kernel_spmd(trace=True) works."""
    if "antenv.axon_hooks" in sys.modules:
        return
    try:
        mod = types.ModuleType("antenv.axon_hooks")
        state = {"hook": None}
        mod.set_axon_ntff_profile_hook = lambda h: state.__setitem__("hook", h)
        mod.get_axon_ntff_profile_hook = lambda: state["hook"]
        sys.modules["antenv.axon_hooks"] = mod
        from trn_agent_boot.trn_boot import _ntff_profile_via_ctypes

        mod.set_axon_ntff_profile_hook(
            _ntff_profile_via_ctypes("/opt/axon/libaxon_pjrt.so")
        )
    except Exception:
        sys.modules.pop("antenv.axon_hooks", None)


def _split_multi_waits(nc):
    """This walrus build accepts at most ONE sem wait per instruction; Tile
    packs several.  Split extras into preceding single-wait NOPs on the same
    engine (equivalent semantics: the engine blocks on them in order)."""
    import bass_rust

    cnt = 0
    for bbw in nc.main_func.blocks:
        bb = bbw.bb if hasattr(bbw, "bb") else bbw
        out = []
        changed = False
        for ins in bb.instructions:
            si = ins.sync_info
            if si is not None and len(si.on_wait) > 1:
                changed = True
                waits = list(si.on_wait)
                for w in waits[:-1]:
                    cnt += 1
                    nop = bass_rust.InstNoOp(name=f"I-wsp{cnt}", ins=[], outs=[])
                    nop.engine = ins.engine
                    nop.sync_info = bass_rust.SyncInfo(on_wait=[w], on_update=[])
                    out.append(nop)
                si.on_wait = [waits[-1]]
                ins.sync_info = si
            out.append(ins)
        if changed:
            bb.instructions = out
    return cnt


def _build_nc(split=True, phase=5):
    from contextlib import ExitStack

    import concourse.bass as bass
    import concourse.tile as tile
    from concourse import mybir

    bf16 = mybir.dt.bfloat16
    f16 = mybir.dt.float16
    f32 = mybir.dt.float32

    nc = bass.Bass()
    xq_h = [
        nc.declare_dram_parameter(f"xq{i}", [128, 8, 1024], bf16, isOutput=False)
        for i in range(2)
    ]
    xk_h = [
        nc.declare_dram_parameter(f"xk{i}", [128, 8, 1024], bf16, isOutput=False)
        for i in range(2)
    ]
    xv_h = [
        nc.declare_dram_parameter(f"xv{i}", [128, 8, 1024], bf16, isOutput=False)
        for i in range(2)
    ]
    wq = nc.declare_dram_parameter("wq", [128, 8 * 256], bf16, isOutput=False)
    wk = nc.declare_dram_parameter("wk", [128, 8 * 256], bf16, isOutput=False)
    wv = nc.declare_dram_parameter("wv", [128, 8 * 260], bf16, isOutput=False)
    wo = nc.declare_dram_parameter("wo", [128, 2 * 1024], bf16, isOutput=False)
    bq = nc.declare_dram_parameter("bq", [128, 2], f32, isOutput=False)
    bk = nc.declare_dram_parameter("bk", [128, 2], f32, isOutput=False)
    bvp = nc.declare_dram_parameter("bvp", [1, 260], f32, isOutput=False)
    identm = nc.declare_dram_parameter("identm", [128, 128], bf16, isOutput=False)
    tneg = nc.declare_dram_parameter("tneg", [128, 128], bf16, isOutput=False)
    outp = nc.declare_dram_parameter("outp", [S, D], f16, isOutput=True)

    with tile.TileContext(nc) as tc, ExitStack() as ctx:
        consts = ctx.enter_context(tc.tile_pool(name="consts", bufs=1))
        xs = ctx.enter_context(tc.tile_pool(name="xs", bufs=6))
        acts = ctx.enter_context(tc.tile_pool(name="acts", bufs=1))
        exps = ctx.enter_context(tc.tile_pool(name="exps", bufs=6))
        rcps = ctx.enter_context(tc.tile_pool(name="rcps", bufs=4))
        osb = ctx.enter_context(tc.tile_pool(name="osb", bufs=4))
        ps_small = ctx.enter_context(
            tc.tile_pool(name="ps_small", bufs=2, space="PSUM")
        )
        ps_sc = ctx.enter_context(tc.tile_pool(name="ps_sc", bufs=2, space="PSUM"))
        ps_av = ctx.enter_context(tc.tile_pool(name="ps_av", bufs=2, space="PSUM"))

        # ---- persistent activation tiles ----
        qt = [acts.tile([128, S], bf16, name=f"qt{m}", tag=f"qt{m}") for m in range(2)]
        kt = [acts.tile([128, S], bf16, name=f"kt{m}", tag=f"kt{m}") for m in range(2)]
        vh_sb = acts.tile([128, 16, 260], bf16, name="vh", tag="vh")
        # attn-out as head PAIRS [2 heads x 64 dk = 128 partitions, S]
        outT = [
            acts.tile([128, S], bf16, name=f"outT{p}", tag=f"outT{p}")
            for p in range(2)
        ]

        def dma2(dst_tile, src):
            """Split a [128, 8, 1024] tensor load into two 1MB DMAs so the
            first accumulation chains can start earlier."""
            nc.sync.dma_start(out=dst_tile[:, 0:4, :], in_=src[:, 0:4, :])
            nc.sync.dma_start(out=dst_tile[:, 4:8, :], in_=src[:, 4:8, :])

        # ---- DMA stream (order == consume order) ----
        wk_sb = consts.tile([128, 8 * 256], bf16)
        nc.sync.dma_start(out=wk_sb[:], in_=wk[:])
        xk_t = []
        for half in range(2):
            t = xs.tile([128, 8, S // 2], bf16, name="xt", tag="xt")
            dma2(t, xk_h[half])
            xk_t.append(t)
        bk_sb = consts.tile([128, 2], f32)
        nc.sync.dma_start(out=bk_sb[:], in_=bk[:])
        wq_sb = consts.tile([128, 8 * 256], bf16, name="wq_sb")
        nc.sync.dma_start(out=wq_sb[:], in_=wq[:])
        bq_sb = consts.tile([128, 2], f32, name="bq_sb")
        nc.sync.dma_start(out=bq_sb[:], in_=bq[:])
        xq_t = [xs.tile([128, 8, S // 2], bf16, name="xt", tag="xt") for _ in range(2)]
        dma2(xq_t[0], xq_h[0])
        id_sb = consts.tile([128, 128], bf16, name="id_sb")
        nc.sync.dma_start(out=id_sb[:], in_=identm[:])
        tn_sb = consts.tile([128, 128], bf16, name="tn_sb")
        nc.sync.dma_start(out=tn_sb[:], in_=tneg[:])
        wv_sb = consts.tile([128, 8 * 260], bf16, name="wv_sb")
        nc.sync.dma_start(out=wv_sb[:], in_=wv[:])
        bvp_sb = consts.tile([128, 260], f32, name="bvp_sb")
        nc.sync.dma_start(out=bvp_sb[:], in_=bvp[:].to_broadcast((128, 260)))
        xv_t = [xs.tile([128, 8, S // 2], bf16, name="xt", tag="xt") for _ in range(2)]
        dma2(xv_t[0], xv_h[0])
        dma2(xq_t[1], xq_h[1])
        dma2(xv_t[1], xv_h[1])
        wo_sb = consts.tile([128, 2 * 1024], bf16, name="wo_sb")
        nc.sync.dma_start(out=wo_sb[:], in_=wo[:])
        ones_sb = consts.tile([65, 64], bf16)
        nc.vector.memset(ones_sb[:], 1.0)

        # ---- projection unit emitters ----
        def kq_unit(xt, wsb, bsb, dst, m, half, scq):
            """One [128,512] output chain of a K/Q projection."""
            sc = half * 2 + scq
            ps = ps_small.tile([128, 512], f32, name="ps", tag="ps")
            for dc in range(8):
                nc.tensor.matmul(
                    ps[:],
                    lhsT=wsb[:, dc * 256 + m * 128: dc * 256 + (m + 1) * 128],
                    rhs=xt[half][:, dc, scq * 512:(scq + 1) * 512],
                    start=(dc == 0),
                    stop=(dc == 7),
                )
            nc.vector.tensor_scalar_add(
                dst[m][:, sc * 512:(sc + 1) * 512], ps[:], bsb[:, m:m + 1]
            )

        def v_unit(st):
            """One s-block (= one kc block) of the V projection."""
            ps = ps_small.tile([128, 512], f32, name="ps", tag="ps")
            for dc in range(8):
                nc.tensor.matmul(
                    ps[:, :260],
                    lhsT=xv_t[st // 8][:, dc, (st % 8) * 128:(st % 8 + 1) * 128],
                    rhs=wv_sb[:, dc * 260:(dc + 1) * 260],
                    start=(dc == 0),
                    stop=(dc == 7),
                )
            nc.vector.tensor_add(vh_sb[:, st, :], ps[:, :260], bvp_sb[:])

        # ---- prologue: K proj pair0 (both halves) + Q proj pair0 half0 ----
        if phase >= 1:
            for half in range(2):
                for scq in range(2):
                    kq_unit(xk_t, wk_sb, bk_sb, kt, 0, half, scq)
            for scq in range(2):
                kq_unit(xq_t, wq_sb, bq_sb, qt, 0, 0, scq)

        # ---- filler queue for pair-0 attention ----
        fillers = deque()
        if phase >= 1:
            for st in (0, 1, 2, 3):
                fillers.append(lambda st=st: v_unit(st))
            for scq in range(2):
                fillers.append(
                    lambda scq=scq: kq_unit(xq_t, wq_sb, bq_sb, qt, 0, 1, scq)
                )
            for st in (4, 5, 6, 7):
                fillers.append(lambda st=st: v_unit(st))
            for half in range(2):
                for scq in range(2):
                    fillers.append(
                        lambda half=half, scq=scq: kq_unit(
                            xk_t, wk_sb, bk_sb, kt, 1, half, scq
                        )
                    )
            for st in (8, 9, 10, 11):
                fillers.append(lambda st=st: v_unit(st))
            for half in range(2):
                for scq in range(2):
                    fillers.append(
                        lambda half=half, scq=scq: kq_unit(
                            xq_t, wq_sb, bq_sb, qt, 1, half, scq
                        )
                    )
            for st in (12, 13, 14, 15):
                fillers.append(lambda st=st: v_unit(st))

        def inject(n):
            for _ in range(n):
                if not fillers:
                    return
                fillers.popleft()()

        # ---- attention ----
        tails = {}  # (pair, qc) -> (posb{h}, rcp{h})

        def trim_c0(qc, kc):
            jr = kc - 4 * qc
            return 128 * jr if jr >= 0 else 0

        def emit_scores_exp(qc, pair, g, exg):
            heads = (2 * pair, 2 * pair + 1)
            for h in heads:
                hr = slice(64 * (h % 2), 64 * (h % 2) + 64)
                pss = ps_sc.tile([128, GK, QC], f32, name="pss", tag="pss")
                for j in range(GK):
                    kc = GK * g + j
                    c0 = trim_c0(qc, kc)
                    diag = kc - 4 * qc >= 0
                    nc.tensor.matmul(
                        pss[:, j, c0:],
                        lhsT=kt[pair][hr, kc * 128:(kc + 1) * 128],
                        rhs=qt[pair][hr, qc * QC + c0:(qc + 1) * QC],
                        start=True,
                        stop=not diag,
                        skip_group_check=True,
                    )
                    if diag:
                        # add -1e9 to the strictly-upper triangle of the
                        # diagonal 128-block: psum += I.T @ Tneg (53ns)
                        nc.tensor.matmul(
                            pss[:, j, c0:c0 + 128],
                            lhsT=id_sb[:],
                            rhs=tn_sb[:],
                            start=False,
                            stop=True,
                            skip_group_check=True,
                        )
                ex = exps.tile([128, GK, QC], bf16, name="ex", tag="ex")
                # full-width exp (2-D contiguous AP); the trimmed-away columns
                # hold stale PSUM whose exp lands in ex columns the (equally
                # trimmed) attn@V matmuls never read.
                nc.scalar.activation(
                    ex[:, :, :], pss[:, :, :], mybir.ActivationFunctionType.Exp
                )
                exg[h] = ex

        def emit_attnv(qc, pair, g, po, last_kc, exg):
            for h in (2 * pair, 2 * pair + 1):
                for j in range(GK):
                    kc = GK * g + j
                    c0 = trim_c0(qc, kc)
                    nc.tensor.matmul(
                        po[h][:, c0:],
                        lhsT=vh_sb[:, kc, h * 65:(h + 1) * 65],
                        rhs=exg[h][:, j, c0:],
                        start=(kc == 0),
                        stop=(kc == last_kc),
                        skip_group_check=True,
                    )

        def emit_pair_tail(qc, pair, po):
            # stage attn-out to SBUF bf16 FIRST so the po PSUM banks free for
            # the next q-chunk with no long ops ahead of them; denominator
            # reciprocal then runs on ScalarE as exp(-ln x) (same table set
            # as the attention exps).
            posb_d, rcp_d, dn_d = {}, {}, {}
            for h in (2 * pair, 2 * pair + 1):
                posb = rcps.tile([64, 512], bf16, name="posb", tag="posb", bufs=8)
                with nc.allow_low_precision(reason="attn-out staged bf16"):
                    nc.vector.tensor_copy(posb[:, :], po[h][0:64, :])
                posb_d[h] = posb
                # stage the denominator row to SBUF too: po's readers are then
                # all immediate DVE copies, so its PSUM banks recycle without
                # waiting on ScalarE's queue
                dn = rcps.tile([65, 512], f32, name="dn", tag="dn", bufs=4)
                nc.vector.tensor_copy(dn[64:65, :], po[h][64:65, :])
                dn_d[h] = dn
            for h in (2 * pair, 2 * pair + 1):
                lg = rcps.tile([65, 512], f32, name="lg", tag="lg", bufs=4)
                nc.scalar.activation(
                    lg[64:65, :],
                    dn_d[h][64:65, :],
                    mybir.ActivationFunctionType.Ln,
                )
                rcp = rcps.tile([65, 512], bf16, name="rcp", tag="rcp", bufs=8)
                nc.scalar.activation(
                    rcp[64:65, :],
                    lg[64:65, :],
                    mybir.ActivationFunctionType.Exp,
                    scale=-1.0,
                )
                rcp_d[h] = rcp
            tails[(pair, qc)] = (posb_d, rcp_d)

        def emit_bc(qc, pair):
            posb_d, rcp_d = tails[(pair, qc)]
            for h in (2 * pair, 2 * pair + 1):
                bc = ps_small.tile([128, 512], f32, name="ps", tag="ps")
                nc.tensor.matmul(
                    bc[0:64, :],
                    lhsT=ones_sb[64:65, :],
                    rhs=rcp_d[h][64:65, :],
                    start=True,
                    stop=True,
                )
                # DVE can't take two PSUM operands; stage via SBUF.  The
                # normalize multiply runs on DVE right behind the copy: no
                # cross-engine hop, and bf16 runs at 2 elem/lane/cycle, so
                # it is both faster and lower-latency than the Pool engine.
                bcs = rcps.tile([64, 512], bf16, name="bcs", tag="bcs", bufs=8)
                nc.vector.tensor_copy(bcs[:, :], bc[0:64, :])
                nc.vector.tensor_mul(
                    outT[h // 2][64 * (h % 2):64 * (h % 2) + 64,
                                 qc * QC:(qc + 1) * QC],
                    posb_d[h][:, :],
                    bcs[:, :],
                )

        def oproj_unit(qc, sti):
            st = qc * 4 + sti
            for ns in range(2):
                ps = ps_small.tile([128, 512], f32, name="ps", tag="ps")
                for hp in range(2):
                    nc.tensor.matmul(
                        ps[:],
                        lhsT=outT[hp][:, st * 128:(st + 1) * 128],
                        rhs=wo_sb[:, hp * 1024 + ns * 512: hp * 1024 + (ns + 1) * 512],
                        start=(hp == 0),
                        stop=(hp == 1),
                    )
                ot = osb.tile([128, 512], f16, name="ot", tag="ot")
                with nc.allow_low_precision(reason="f16 partials"):
                    nc.vector.tensor_copy(ot[:], ps[:])
                nc.sync.dma_start(
                    out=outp[st * 128:(st + 1) * 128, ns * 512:(ns + 1) * 512],
                    in_=ot[:],
                )

        if phase >= 3:
            for pair in range(2):
                for qc in range(NQC):
                    po = {}
                    for h in (2 * pair, 2 * pair + 1):
                        po[h] = ps_av.tile([65, 512], f32, name="po", tag="po")
                    ngroups = GK * (qc + 1)
                    last_kc = 4 * qc + 3
                    prev = None
                    for g in range(ngroups):
                        exg = {}
                        emit_scores_exp(qc, pair, g, exg)
                        inject(2 if pair == 0 else 1)
                        if prev is not None:
                            emit_attnv(qc, pair, prev[0], po, last_kc, prev[1])
                        prev = (g, exg)
                    emit_attnv(qc, pair, prev[0], po, last_kc, prev[1])
                    emit_pair_tail(qc, pair, po)
                    if pair == 1 and phase >= 4:
                        # both pairs of this qc are done: normalize pair1 now,
                        # queue this qc's oproj as fillers for the next chunk
                        emit_bc(qc, 1)
                        if phase >= 5:
                            for sti in range(4):
                                fillers.append(
                                    lambda qc=qc, sti=sti: oproj_unit(qc, sti)
                                )
                if pair == 0 and phase >= 4:
                    # while pair1's attention runs, normalize pair0's heads
                    for qc in range(NQC):
                        fillers.append(lambda qc=qc: emit_bc(qc, 0))
            # drain remaining fillers (late oprojs)
            inject(len(fillers))

        if phase < 5:
            ot = osb.tile([128, 512], f16, name="ot", tag="ot")
            nc.vector.memset(ot[:], 0.0)
            nc.sync.dma_start(out=outp[0:128, 0:512], in_=ot[:])

    if split:
        _split_multi_waits(nc)
    return nc


_NC_CACHE = None


def _get_nc():
    global _NC_CACHE
    if _NC_CACHE is None:
        _NC_CACHE = _build_nc()
    return _NC_CACHE


def _swizzle_w(wT, block):
    """wT [D, C] -> [128, 8*C] so that out[p, dc*C + j] = wT[dc*128 + p, j]."""
    dcs = wT.shape[0] // 128
    return np.ascontiguousarray(
        wT.reshape(dcs, 128, wT.shape[1]).transpose(1, 0, 2).reshape(128, -1)
    )


def _np_reference(q, k, v, mask, Wq, bq, Wk, bk, Wv, bv, Wo, bo):
    def split_heads(x):
        b, s, _ = x.shape
        return x.reshape(b, s, H, DK).transpose(0, 2, 1, 3)

    qh = split_heads(q @ Wq.T + bq)
    kh = split_heads(k @ Wk.T + bk)
    vh = split_heads(v @ Wv.T + bv)
    scores = np.einsum("bhqd,bhkd->bhqk", qh, kh) / np.sqrt(np.float32(DK))
    scores = np.where(mask, np.float32(-1e9), scores)
    scores = scores - scores.max(axis=-1, keepdims=True)
    e = np.exp(scores)
    attn = e / e.sum(axis=-1, keepdims=True)
    out = np.einsum("bhqk,bhkd->bhqd", attn, vh)
    out = out.transpose(0, 2, 1, 3).reshape(q.shape[0], -1, D)
    return (out @ Wo.T + bo).astype(np.float32)


def kernel(q, k, v, mask, Wq, bq, Wk, bk, Wv, bv, Wo, bo):
    import ml_dtypes

    bf16 = ml_dtypes.bfloat16

    q = np.asarray(q, np.float32)
    k = np.asarray(k, np.float32)
    v = np.asarray(v, np.float32)
    mask = np.asarray(mask, bool)
    Wq = np.asarray(Wq, np.float32)
    bq = np.asarray(bq, np.float32)
    Wk = np.asarray(Wk, np.float32)
    bk = np.asarray(bk, np.float32)
    Wv = np.asarray(Wv, np.float32)
    bv = np.asarray(bv, np.float32)
    Wo = np.asarray(Wo, np.float32)
    bo = np.asarray(bo, np.float32)

    causal = np.triu(np.ones((S, S), dtype=bool), k=1)
    if not np.array_equal(mask.reshape(S, S), causal):
        return _np_reference(q, k, v, mask, Wq, bq, Wk, bk, Wv, bv, Wo, bo)

    _install_ntff_hook()
    from concourse.bass_utils import run_bass_kernel_spmd

    nc = _get_nc()

    kk = np.arange(128)[:, None]
    qq = np.arange(128)[None, :]
    tneg_m = np.where(kk > qq, np.float32(-1e9), np.float32(0)).astype(bf16)
    ident_m = np.eye(128, dtype=np.float32).astype(bf16)

    # x.T [D, S] -> per half [128, 8, 1024] with x_h[p, dc, s] =
    # xT[dc*128 + p, half*1024 + s]; contiguous 16KB per partition.
    xT = {}
    for name, x in (("q", q), ("k", k), ("v", v)):
        per_b = []
        for b in range(B):
            xt = x[b].T.astype(bf16).reshape(8, 128, 2048)
            per_b.append(
                [
                    np.ascontiguousarray(
                        xt[:, :, hf * 1024:(hf + 1) * 1024].transpose(1, 0, 2)
                    )
                    for hf in range(2)
                ]
            )
        xT[name] = per_b

    in_maps = []
    for c in range(N_CORES):
        b = c // 4
        g = c % 4
        hs = slice(g * HPC * DK, (g + 1) * HPC * DK)  # 256 rows of W, cols of Wo
        wq_c = _swizzle_w((SCALE * Wq[hs]).T.astype(bf16), 256)
        wk_c = _swizzle_w(Wk[hs].T.astype(bf16), 256)
        # V' with a zero weight column at h*65+64 (ones come via bias row)
        wvT = Wv[hs].T  # [1024, 256]
        wvp = np.zeros((D, 260), np.float32)
        for h in range(HPC):
            wvp[:, h * 65:h * 65 + 64] = wvT[:, h * 64:(h + 1) * 64]
        wv_c = _swizzle_w(wvp.astype(bf16), 260)
        # wo: (Wo.T)[hs, :] [256, 1024] -> head-pair blocks [128, 2*1024]
        woT = np.ascontiguousarray(Wo[:, hs].T)
        wo_c = np.ascontiguousarray(
            woT.reshape(2, 128, 1024).transpose(1, 0, 2).reshape(128, 2048)
        ).astype(bf16)
        bq_c = np.ascontiguousarray(
            (SCALE * bq[hs]).reshape(2, 128).T.astype(np.float32)
        )
        bk_c = np.ascontiguousarray(bk[hs].reshape(2, 128).T.astype(np.float32))
        bvp_c = np.zeros((1, 260), np.float32)
        for h in range(HPC):
            bvp_c[0, h * 65:h * 65 + 64] = bv[hs][h * 64:(h + 1) * 64]
            bvp_c[0, h * 65 + 64] = 1.0
        in_maps.append(
            {
                "xq0": xT["q"][b][0],
                "xq1": xT["q"][b][1],
                "xk0": xT["k"][b][0],
                "xk1": xT["k"][b][1],
                "xv0": xT["v"][b][0],
                "xv1": xT["v"][b][1],
                "wq": wq_c,
                "wk": wk_c,
                "wv": wv_c,
                "wo": wo_c,
                "bq": bq_c,
                "bk": bk_c,
                "bvp": bvp_c,
                "identm": ident_m,
                "tneg": tneg_m,
            }
        )

    trace = bool(os.environ.get("BASSMHA_TRACE"))
    res = run_bass_kernel_spmd(nc, in_maps, list(range(N_CORES)), trace=trace)
    kernel._last_exec_ns = res.exec_time_ns
    kernel._last_mean_exec_ns = res.mean_exec_time_ns

    out = np.zeros((B, S, D), np.float64)
    for c in range(N_CORES):
        out[c // 4] += res.results[c]["outp"].astype(np.float64)
    out += bo.astype(np.float64)
    return out.astype(np.float32)


# revision 33
# speedup vs baseline: 1.0583x; 1.0192x over previous
"""Multi-head attention (B=2, S=2048, D=1024, H=16, causal) on 8 Trainium2
NeuronCores via Bass/Tile.

Sharding: core c -> batch c//4, heads [4*(c%4), 4*(c%4)+4)  (DP over batch x
TP over heads).  QKV weights column-parallel, O row-parallel; the 4 partial
[S, D] outputs per batch are summed on the host (gather step), bias bo added
there too.

Per-core dataflow (bf16 matmuls, fp32 PSUM accumulation):
  - host supplies x.T per batch in a dc-swizzled layout ([p, dc, s], 16KB
    contiguous per partition) so input DMA runs at full bandwidth.
  - Q/K proj -> qT/kT [dk_c=256, S] (head-major, 2 chunks of 128 = 2 heads);
    V proj -> natural [S, 260]: per head 64 cols of V plus a ones column for
    the flash-style softmax denominator.
  - scores transposed: sT[k, q] = kT.T @ qT per head; causal masking is done
    ON THE PE: diagonal 128-blocks get a second accumulating matmul
    (identity.T @ Tneg) that adds -1e9 to the strictly-upper triangle, so
    exp() zeroes it with no vector-engine hop; strictly-upper blocks are
    skipped and diagonal-band matmuls are column-trimmed.
  - exp on ScalarE reads scores PSUM directly; attn@V: outT[65, q] +=
    V'[k,65].T @ expT[k,q]; row 64 accumulates the denominator.
  - normalize: DVE reciprocal of the denominator row + PE outer-product
    broadcast + Pool-engine multiply into head-PAIR tiles [128, S], so the
    O projection contracts 128 partitions (2 heads x 64 dk) per matmul.
  - PSUM -> f16 SBUF -> DRAM f16 partials.

Schedule: PAIR-MAJOR with filler injection.  Prologue: K proj (pair 0) +
Q proj (pair 0, S-half 0) while DMAs stream.  Attention for pair 0 then
starts ~25us in; the remaining projection work (V per-kc units, pair-1 K/Q,
pair-0 Q half 1) is injected one unit at a time between attention groups so
the PE never idles while ScalarE exps drain.  During pair 1's attention the
fillers are pair-0's bc/normalize and the O projection of finished q-chunks.
Attention groups are software-pipelined one group deep (attn@V of group g-1
issues after scores+exp of group g).
"""

import os
import sys
import types
from collections import deque

import numpy as np

B, S, D, H = 2, 2048, 1024, 16
DK = D // H  # 64
N_CORES = 8
HPC = 4  # heads per core
SCALE = 1.0 / np.sqrt(np.float32(DK))  # folded into Wq/bq on host

QC = 512  # query block (free dim of scores matmuls)
NQC = S // QC  # 4
GK = 2  # key blocks per exp group -> scores psum tile [128, GK, QC]


def _install_ntff_hook():
    """The image's antenv lacks axon_hooks; register the NTFF profile hook
    ourselves so run_bass_kernel_spmd(trace=True) works."""
    if "antenv.axon_hooks" in sys.modules:
        return
    try:
        mod = types.ModuleType("antenv.axon_hooks")
        state = {"hook": None}
        mod.set_axon_ntff_profile_hook = lambda h: state.__setitem__("hook", h)
        mod.get_axon_ntff_profile_hook = lambda: state["hook"]
        sys.modules["antenv.axon_hooks"] = mod
        from trn_agent_boot.trn_boot import _ntff_profile_via_ctypes

        mod.set_axon_ntff_profile_hook(
            _ntff_profile_via_ctypes("/opt/axon/libaxon_pjrt.so")
        )
    except Exception:
        sys.modules.pop("antenv.axon_hooks", None)


def _split_multi_waits(nc):
    """This walrus build accepts at most ONE sem wait per instruction; Tile
    packs several.  Split extras into preceding single-wait NOPs on the same
    engine (equivalent semantics: the engine blocks on them in order)."""
    import bass_rust

    cnt = 0
    for bbw in nc.main_func.blocks:
        bb = bbw.bb if hasattr(bbw, "bb") else bbw
        out = []
        changed = False
        for ins in bb.instructions:
            si = ins.sync_info
            if si is not None and len(si.on_wait) > 1:
                changed = True
                waits = list(si.on_wait)
                for w in waits[:-1]:
                    cnt += 1
                    nop = bass_rust.InstNoOp(name=f"I-wsp{cnt}", ins=[], outs=[])
                    nop.engine = ins.engine
                    nop.sync_info = bass_rust.SyncInfo(on_wait=[w], on_update=[])
                    out.append(nop)
                si.on_wait = [waits[-1]]
                ins.sync_info = si
            out.append(ins)
        if changed:
            bb.instructions = out
    return cnt


def _build_nc(split=True, phase=5):
    from contextlib import ExitStack

    import concourse.bass as bass
    import concourse.tile as tile
    from concourse import mybir

    bf16 = mybir.dt.bfloat16
    f16 = mybir.dt.float16
    f32 = mybir.dt.float32

    nc = bass.Bass()
    xq_h = [
        nc.declare_dram_parameter(f"xq{i}", [128, 8, 1024], bf16, isOutput=False)
        for i in range(2)
    ]
    xk_h = [
        nc.declare_dram_parameter(f"xk{i}", [128, 8, 1024], bf16, isOutput=False)
        for i in range(2)
    ]
    xv_h = [
        nc.declare_dram_parameter(f"xv{i}", [128, 8, 1024], bf16, isOutput=False)
        for i in range(2)
    ]
    wq = nc.declare_dram_parameter("wq", [128, 8 * 256], bf16, isOutput=False)
    wk = nc.declare_dram_parameter("wk", [128, 8 * 256], bf16, isOutput=False)
    wv = nc.declare_dram_parameter("wv", [128, 8 * 260], bf16, isOutput=False)
    wo = nc.declare_dram_parameter("wo", [128, 2 * 1024], bf16, isOutput=False)
    bq = nc.declare_dram_parameter("bq", [128, 2], f32, isOutput=False)
    bk = nc.declare_dram_parameter("bk", [128, 2], f32, isOutput=False)
    bvp = nc.declare_dram_parameter("bvp", [1, 260], f32, isOutput=False)
    identm = nc.declare_dram_parameter("identm", [128, 128], bf16, isOutput=False)
    tneg = nc.declare_dram_parameter("tneg", [128, 128], bf16, isOutput=False)
    outp = nc.declare_dram_parameter("outp", [S, D], f16, isOutput=True)

    with tile.TileContext(nc) as tc, ExitStack() as ctx:
        consts = ctx.enter_context(tc.tile_pool(name="consts", bufs=1))
        xs = ctx.enter_context(tc.tile_pool(name="xs", bufs=6))
        acts = ctx.enter_context(tc.tile_pool(name="acts", bufs=1))
        exps = ctx.enter_context(tc.tile_pool(name="exps", bufs=6))
        rcps = ctx.enter_context(tc.tile_pool(name="rcps", bufs=4))
        osb = ctx.enter_context(tc.tile_pool(name="osb", bufs=4))
        ps_small = ctx.enter_context(
            tc.tile_pool(name="ps_small", bufs=2, space="PSUM")
        )
        ps_sc = ctx.enter_context(tc.tile_pool(name="ps_sc", bufs=2, space="PSUM"))
        ps_av = ctx.enter_context(tc.tile_pool(name="ps_av", bufs=2, space="PSUM"))

        # ---- persistent activation tiles ----
        qt = [acts.tile([128, S], bf16, name=f"qt{m}", tag=f"qt{m}") for m in range(2)]
        kt = [acts.tile([128, S], bf16, name=f"kt{m}", tag=f"kt{m}") for m in range(2)]
        vh_sb = acts.tile([128, 16, 260], bf16, name="vh", tag="vh")
        # attn-out as head PAIRS [2 heads x 64 dk = 128 partitions, S]
        outT = [
            acts.tile([128, S], bf16, name=f"outT{p}", tag=f"outT{p}")
            for p in range(2)
        ]

        def dma2(dst_tile, src):
            """Split a [128, 8, 1024] tensor load into two 1MB DMAs so the
            first accumulation chains can start earlier."""
            nc.sync.dma_start(out=dst_tile[:, 0:4, :], in_=src[:, 0:4, :])
            nc.sync.dma_start(out=dst_tile[:, 4:8, :], in_=src[:, 4:8, :])

        # ---- DMA stream (order == consume order) ----
        wk_sb = consts.tile([128, 8 * 256], bf16)
        nc.sync.dma_start(out=wk_sb[:], in_=wk[:])
        xk_t = []
        for half in range(2):
            t = xs.tile([128, 8, S // 2], bf16, name="xt", tag="xt")
            dma2(t, xk_h[half])
            xk_t.append(t)
        bk_sb = consts.tile([128, 2], f32)
        nc.sync.dma_start(out=bk_sb[:], in_=bk[:])
        wq_sb = consts.tile([128, 8 * 256], bf16, name="wq_sb")
        nc.sync.dma_start(out=wq_sb[:], in_=wq[:])
        bq_sb = consts.tile([128, 2], f32, name="bq_sb")
        nc.sync.dma_start(out=bq_sb[:], in_=bq[:])
        xq_t = [xs.tile([128, 8, S // 2], bf16, name="xt", tag="xt") for _ in range(2)]
        dma2(xq_t[0], xq_h[0])
        id_sb = consts.tile([128, 128], bf16, name="id_sb")
        nc.sync.dma_start(out=id_sb[:], in_=identm[:])
        tn_sb = consts.tile([128, 128], bf16, name="tn_sb")
        nc.sync.dma_start(out=tn_sb[:], in_=tneg[:])
        wv_sb = consts.tile([128, 8 * 260], bf16, name="wv_sb")
        nc.sync.dma_start(out=wv_sb[:], in_=wv[:])
        bvp_sb = consts.tile([128, 260], f32, name="bvp_sb")
        nc.sync.dma_start(out=bvp_sb[:], in_=bvp[:].to_broadcast((128, 260)))
        xv_t = [xs.tile([128, 8, S // 2], bf16, name="xt", tag="xt") for _ in range(2)]
        dma2(xv_t[0], xv_h[0])
        dma2(xq_t[1], xq_h[1])
        dma2(xv_t[1], xv_h[1])
        wo_sb = consts.tile([128, 2 * 1024], bf16, name="wo_sb")
        nc.sync.dma_start(out=wo_sb[:], in_=wo[:])
        ones_sb = consts.tile([65, 64], bf16)
        nc.vector.memset(ones_sb[:], 1.0)

        # ---- projection unit emitters ----
        def kq_unit(xt, wsb, bsb, dst, m, half, scq):
            """One [128,512] output chain of a K/Q projection."""
            sc = half * 2 + scq
            ps = ps_small.tile([128, 512], f32, name="ps", tag="ps")
            for dc in range(8):
                nc.tensor.matmul(
                    ps[:],
                    lhsT=wsb[:, dc * 256 + m * 128: dc * 256 + (m + 1) * 128],
                    rhs=xt[half][:, dc, scq * 512:(scq + 1) * 512],
                    start=(dc == 0),
                    stop=(dc == 7),
                )
            nc.vector.tensor_scalar_add(
                dst[m][:, sc * 512:(sc + 1) * 512], ps[:], bsb[:, m:m + 1]
            )

        def v_unit(st):
            """One s-block (= one kc block) of the V projection."""
            ps = ps_small.tile([128, 512], f32, name="ps", tag="ps")
            for dc in range(8):
                nc.tensor.matmul(
                    ps[:, :260],
                    lhsT=xv_t[st // 8][:, dc, (st % 8) * 128:(st % 8 + 1) * 128],
                    rhs=wv_sb[:, dc * 260:(dc + 1) * 260],
                    start=(dc == 0),
                    stop=(dc == 7),
                )
            nc.vector.tensor_add(vh_sb[:, st, :], ps[:, :260], bvp_sb[:])

        # ---- prologue: K proj pair0 (both halves) + Q proj pair0 half0 ----
        if phase >= 1:
            for half in range(2):
                for scq in range(2):
                    kq_unit(xk_t, wk_sb, bk_sb, kt, 0, half, scq)
            for scq in range(2):
                kq_unit(xq_t, wq_sb, bq_sb, qt, 0, 0, scq)

        # ---- filler queue for pair-0 attention ----
        fillers = deque()
        if phase >= 1:
            for st in (0, 1, 2, 3):
                fillers.append(lambda st=st: v_unit(st))
            for scq in range(2):
                fillers.append(
                    lambda scq=scq: kq_unit(xq_t, wq_sb, bq_sb, qt, 0, 1, scq)
                )
            for st in (4, 5, 6, 7):
                fillers.append(lambda st=st: v_unit(st))
            for half in range(2):
                for scq in range(2):
                    fillers.append(
                        lambda half=half, scq=scq: kq_unit(
                            xk_t, wk_sb, bk_sb, kt, 1, half, scq
                        )
                    )
            for st in (8, 9, 10, 11):
                fillers.append(lambda st=st: v_unit(st))
            for half in range(2):
                for scq in range(2):
                    fillers.append(
                        lambda half=half, scq=scq: kq_unit(
                            xq_t, wq_sb, bq_sb, qt, 1, half, scq
                        )
                    )
            for st in (12, 13, 14, 15):
                fillers.append(lambda st=st: v_unit(st))

        def inject(n):
            for _ in range(n):
                if not fillers:
                    return
                fillers.popleft()()

        # ---- attention ----
        tails = {}  # (pair, qc) -> {posb, dn, rcp}
        pending_recips = []

        def trim_c0(qc, kc):
            jr = kc - 4 * qc
            return 128 * jr if jr >= 0 else 0

        def emit_scores_exp(qc, pair, g, exg):
            heads = (2 * pair, 2 * pair + 1)
            for h in heads:
                hr = slice(64 * (h % 2), 64 * (h % 2) + 64)
                pss = ps_sc.tile([128, GK, QC], f32, name="pss", tag="pss")
                for j in range(GK):
                    kc = GK * g + j
                    c0 = trim_c0(qc, kc)
                    diag = kc - 4 * qc >= 0
                    nc.tensor.matmul(
                        pss[:, j, c0:],
                        lhsT=kt[pair][hr, kc * 128:(kc + 1) * 128],
                        rhs=qt[pair][hr, qc * QC + c0:(qc + 1) * QC],
                        start=True,
                        stop=not diag,
                        skip_group_check=True,
                    )
                    if diag:
                        # add -1e9 to the strictly-upper triangle of the
                        # diagonal 128-block: psum += I.T @ Tneg (53ns)
                        nc.tensor.matmul(
                            pss[:, j, c0:c0 + 128],
                            lhsT=id_sb[:],
                            rhs=tn_sb[:],
                            start=False,
                            stop=True,
                            skip_group_check=True,
                        )
                ex = exps.tile([128, GK, QC], bf16, name="ex", tag="ex")
                # full-width exp (2-D contiguous AP); the trimmed-away columns
                # hold stale PSUM whose exp lands in ex columns the (equally
                # trimmed) attn@V matmuls never read.
                nc.scalar.activation(
                    ex[:, :, :], pss[:, :, :], mybir.ActivationFunctionType.Exp
                )
                exg[h] = ex

        def emit_attnv(qc, pair, g, po, last_kc, exg):
            for h in (2 * pair, 2 * pair + 1):
                for j in range(GK):
                    kc = GK * g + j
                    c0 = trim_c0(qc, kc)
                    nc.tensor.matmul(
                        po[h][:, c0:],
                        lhsT=vh_sb[:, kc, h * 65:(h + 1) * 65],
                        rhs=exg[h][:, j, c0:],
                        start=(kc == 0),
                        stop=(kc == last_kc),
                        skip_group_check=True,
                    )

        def emit_pair_tail(qc, pair, po):
            # DVE-only tail: stage attn-out AND the denominator row to SBUF
            # so the po PSUM banks recycle immediately and no ScalarE work is
            # queued at the q-chunk boundary (where the next chunk's first
            # exp must issue promptly).
            posb_d, dn_d = {}, {}
            for h in (2 * pair, 2 * pair + 1):
                posb = rcps.tile([64, 512], bf16, name="posb", tag="posb", bufs=8)
                with nc.allow_low_precision(reason="attn-out staged bf16"):
                    nc.vector.tensor_copy(posb[:, :], po[h][0:64, :])
                posb_d[h] = posb
                dn = rcps.tile([65, 512], f32, name="dn", tag="dn", bufs=4)
                nc.vector.tensor_copy(dn[64:65, :], po[h][64:65, :])
                dn_d[h] = dn
            tails[(pair, qc)] = {"posb": posb_d, "dn": dn_d, "rcp": {}}
            pending_recips.append((pair, qc))

        def emit_recips():
            # denominator reciprocals (exp(-ln x) on ScalarE) for finished
            # q-chunks, emitted mid-stream one chunk later so they never
            # delay a boundary exp
            while pending_recips:
                pair, qc = pending_recips.pop(0)
                t = tails[(pair, qc)]
                for h in (2 * pair, 2 * pair + 1):
                    lg = rcps.tile([65, 512], f32, name="lg", tag="lg", bufs=4)
                    nc.scalar.activation(
                        lg[64:65, :],
                        t["dn"][h][64:65, :],
                        mybir.ActivationFunctionType.Ln,
                    )
                    rcp = rcps.tile([65, 512], bf16, name="rcp", tag="rcp", bufs=8)
                    nc.scalar.activation(
                        rcp[64:65, :],
                        lg[64:65, :],
                        mybir.ActivationFunctionType.Exp,
                        scale=-1.0,
                    )
                    t["rcp"][h] = rcp

        def emit_bc(qc, pair):
            t = tails[(pair, qc)]
            posb_d, rcp_d = t["posb"], t["rcp"]
            for h in (2 * pair, 2 * pair + 1):
                bc = ps_small.tile([128, 512], f32, name="ps", tag="ps")
                nc.tensor.matmul(
                    bc[0:64, :],
                    lhsT=ones_sb[64:65, :],
                    rhs=rcp_d[h][64:65, :],
                    start=True,
                    stop=True,
                )
                # DVE can't take two PSUM operands; stage via SBUF, then the
                # normalize multiply runs on the otherwise-idle Pool engine.
                bcs = rcps.tile([64, 512], bf16, name="bcs", tag="bcs", bufs=8)
                nc.vector.tensor_copy(bcs[:, :], bc[0:64, :])
                nc.gpsimd.tensor_mul(
                    outT[h // 2][64 * (h % 2):64 * (h % 2) + 64,
                                 qc * QC:(qc + 1) * QC],
                    posb_d[h][:, :],
                    bcs[:, :],
                )

        def oproj_unit(qc, sti):
            st = qc * 4 + sti
            for ns in range(2):
                ps = ps_small.tile([128, 512], f32, name="ps", tag="ps")
                for hp in range(2):
                    nc.tensor.matmul(
                        ps[:],
                        lhsT=outT[hp][:, st * 128:(st + 1) * 128],
                        rhs=wo_sb[:, hp * 1024 + ns * 512: hp * 1024 + (ns + 1) * 512],
                        start=(hp == 0),
                        stop=(hp == 1),
                    )
                ot = osb.tile([128, 512], f16, name="ot", tag="ot")
                with nc.allow_low_precision(reason="f16 partials"):
                    nc.vector.tensor_copy(ot[:], ps[:])
                nc.sync.dma_start(
                    out=outp[st * 128:(st + 1) * 128, ns * 512:(ns + 1) * 512],
                    in_=ot[:],
                )

        if phase >= 3:
            for pair in range(2):
                for qc in range(NQC):
                    po = {}
                    for h in (2 * pair, 2 * pair + 1):
                        po[h] = ps_av.tile([65, 512], f32, name="po", tag="po")
                    ngroups = GK * (qc + 1)
                    last_kc = 4 * qc + 3
                    prev = None
                    for g in range(ngroups):
                        exg = {}
                        emit_scores_exp(qc, pair, g, exg)
                        if g == 0:
                            # recips of the previous chunk: after this chunk's
                            # first exp, before any filler consumes them
                            emit_recips()
                        inject(2 if pair == 0 else 1)
                        if prev is not None:
                            emit_attnv(qc, pair, prev[0], po, last_kc, prev[1])
                        prev = (g, exg)
                    emit_attnv(qc, pair, prev[0], po, last_kc, prev[1])
                    emit_pair_tail(qc, pair, po)
                    if pair == 1 and phase >= 4:
                        # both pairs of this qc are done: queue pair1's bc and
                        # this qc's oproj as next-chunk fillers (the rcp they
                        # consume is emitted at the next chunk's start)
                        fillers.append(lambda qc=qc: emit_bc(qc, 1))
                        if phase >= 5:
                            for sti in range(4):
                                fillers.append(
                                    lambda qc=qc, sti=sti: oproj_unit(qc, sti)
                                )
                if pair == 0 and phase >= 4:
                    # while pair1's attention runs, normalize pair0's heads
                    for qc in range(NQC):
                        fillers.append(lambda qc=qc: emit_bc(qc, 0))
            # drain: last chunk's recips, then remaining bc/oproj fillers
            emit_recips()
            inject(len(fillers))

        if phase < 5:
            ot = osb.tile([128, 512], f16, name="ot", tag="ot")
            nc.vector.memset(ot[:], 0.0)
            nc.sync.dma_start(out=outp[0:128, 0:512], in_=ot[:])

    if split:
        _split_multi_waits(nc)
    return nc


_NC_CACHE = None


def _get_nc():
    global _NC_CACHE
    if _NC_CACHE is None:
        _NC_CACHE = _build_nc()
    return _NC_CACHE


def _swizzle_w(wT, block):
    """wT [D, C] -> [128, 8*C] so that out[p, dc*C + j] = wT[dc*128 + p, j]."""
    dcs = wT.shape[0] // 128
    return np.ascontiguousarray(
        wT.reshape(dcs, 128, wT.shape[1]).transpose(1, 0, 2).reshape(128, -1)
    )


def _np_reference(q, k, v, mask, Wq, bq, Wk, bk, Wv, bv, Wo, bo):
    def split_heads(x):
        b, s, _ = x.shape
        return x.reshape(b, s, H, DK).transpose(0, 2, 1, 3)

    qh = split_heads(q @ Wq.T + bq)
    kh = split_heads(k @ Wk.T + bk)
    vh = split_heads(v @ Wv.T + bv)
    scores = np.einsum("bhqd,bhkd->bhqk", qh, kh) / np.sqrt(np.float32(DK))
    scores = np.where(mask, np.float32(-1e9), scores)
    scores = scores - scores.max(axis=-1, keepdims=True)
    e = np.exp(scores)
    attn = e / e.sum(axis=-1, keepdims=True)
    out = np.einsum("bhqk,bhkd->bhqd", attn, vh)
    out = out.transpose(0, 2, 1, 3).reshape(q.shape[0], -1, D)
    return (out @ Wo.T + bo).astype(np.float32)


def kernel(q, k, v, mask, Wq, bq, Wk, bk, Wv, bv, Wo, bo):
    import ml_dtypes

    bf16 = ml_dtypes.bfloat16

    q = np.asarray(q, np.float32)
    k = np.asarray(k, np.float32)
    v = np.asarray(v, np.float32)
    mask = np.asarray(mask, bool)
    Wq = np.asarray(Wq, np.float32)
    bq = np.asarray(bq, np.float32)
    Wk = np.asarray(Wk, np.float32)
    bk = np.asarray(bk, np.float32)
    Wv = np.asarray(Wv, np.float32)
    bv = np.asarray(bv, np.float32)
    Wo = np.asarray(Wo, np.float32)
    bo = np.asarray(bo, np.float32)

    causal = np.triu(np.ones((S, S), dtype=bool), k=1)
    if not np.array_equal(mask.reshape(S, S), causal):
        return _np_reference(q, k, v, mask, Wq, bq, Wk, bk, Wv, bv, Wo, bo)

    _install_ntff_hook()
    from concourse.bass_utils import run_bass_kernel_spmd

    nc = _get_nc()

    kk = np.arange(128)[:, None]
    qq = np.arange(128)[None, :]
    tneg_m = np.where(kk > qq, np.float32(-1e9), np.float32(0)).astype(bf16)
    ident_m = np.eye(128, dtype=np.float32).astype(bf16)

    # x.T [D, S] -> per half [128, 8, 1024] with x_h[p, dc, s] =
    # xT[dc*128 + p, half*1024 + s]; contiguous 16KB per partition.
    xT = {}
    for name, x in (("q", q), ("k", k), ("v", v)):
        per_b = []
        for b in range(B):
            xt = x[b].T.astype(bf16).reshape(8, 128, 2048)
            per_b.append(
                [
                    np.ascontiguousarray(
                        xt[:, :, hf * 1024:(hf + 1) * 1024].transpose(1, 0, 2)
                    )
                    for hf in range(2)
                ]
            )
        xT[name] = per_b

    in_maps = []
    for c in range(N_CORES):
        b = c // 4
        g = c % 4
        hs = slice(g * HPC * DK, (g + 1) * HPC * DK)  # 256 rows of W, cols of Wo
        wq_c = _swizzle_w((SCALE * Wq[hs]).T.astype(bf16), 256)
        wk_c = _swizzle_w(Wk[hs].T.astype(bf16), 256)
        # V' with a zero weight column at h*65+64 (ones come via bias row)
        wvT = Wv[hs].T  # [1024, 256]
        wvp = np.zeros((D, 260), np.float32)
        for h in range(HPC):
            wvp[:, h * 65:h * 65 + 64] = wvT[:, h * 64:(h + 1) * 64]
        wv_c = _swizzle_w(wvp.astype(bf16), 260)
        # wo: (Wo.T)[hs, :] [256, 1024] -> head-pair blocks [128, 2*1024]
        woT = np.ascontiguousarray(Wo[:, hs].T)
        wo_c = np.ascontiguousarray(
            woT.reshape(2, 128, 1024).transpose(1, 0, 2).reshape(128, 2048)
        ).astype(bf16)
        bq_c = np.ascontiguousarray(
            (SCALE * bq[hs]).reshape(2, 128).T.astype(np.float32)
        )
        bk_c = np.ascontiguousarray(bk[hs].reshape(2, 128).T.astype(np.float32))
        bvp_c = np.zeros((1, 260), np.float32)
        for h in range(HPC):
            bvp_c[0, h * 65:h * 65 + 64] = bv[hs][h * 64:(h + 1) * 64]
            bvp_c[0, h * 65 + 64] = 1.0
        in_maps.append(
            {
                "xq0": xT["q"][b][0],
                "xq1": xT["q"][b][1],
                "xk0": xT["k"][b][0],
                "xk1": xT["k"][b][1],
                "xv0": xT["v"][b][0],
                "xv1": xT["v"][b][1],
                "wq": wq_c,
                "wk": wk_c,
                "wv": wv_c,
                "wo": wo_c,
                "bq": bq_c,
                "bk": bk_c,
                "bvp": bvp_c,
                "identm": ident_m,
                "tneg": tneg_m,
            }
        )

    trace = bool(os.environ.get("BASSMHA_TRACE"))
    res = run_bass_kernel_spmd(nc, in_maps, list(range(N_CORES)), trace=trace)
    kernel._last_exec_ns = res.exec_time_ns
    kernel._last_mean_exec_ns = res.mean_exec_time_ns

    out = np.zeros((B, S, D), np.float64)
    for c in range(N_CORES):
        out[c // 4] += res.results[c]["outp"].astype(np.float64)
    out += bo.astype(np.float64)
    return out.astype(np.float32)
